# revision 25
# baseline (speedup 1.0000x reference)
"""Trainium2 Bass kernel for nn_ADRC_PE (dense CNN: 1x1 reduce -> GroupNorm ->
fixed 3x3 depthwise convs -> curvature gate -> fuse -> residual scale).

Sharding: pure data parallel, batch dim (B=8) across 8 NeuronCores.
Per-core field layout: 128 partitions = 64 channels x 2 row-halves
(partition c holds image rows 0..79 of channel c, partition 64+c rows
80..159), fp16, row-major [hl, w] with 1-row halos and zero padding.
"""

import numpy as np

import concourse.bass as bass
import concourse.tile as tile
from concourse import bacc, mybir
from concourse.bass_utils import run_bass_kernel_spmd

F32 = mybir.dt.float32
F16 = mybir.dt.float16

B, C, H, W = 8, 256, 160, 160
CRED, GROUPS = 64, 8
EPS, GN_EPS = 1e-4, 1e-5

HL = 82          # local h extent per half-block (1 halo/pad row each side)
WP = 162         # padded w extent
NPIX = H * W     # 25600
CH = 10          # phase-B chunk: output rows per chunk (per half-block)
NCHUNK = 80 // CH
AH = 4           # phase-A rows per half-block per iteration
NAIT = 80 // AH  # 20


def _sel8_const():
    """[8, 128] fp32: sel8[g, p] = 1 if channel-group of p == g (broadcast)."""
    s = np.zeros((8, 128), np.float32)
    for p in range(128):
        s[(p % 64) // 8, p] = 1.0
    return s


def _selg_const():
    """[64, 8] fp32: selg[c, g] = 1 if group of channel c == g (group sums)."""
    s = np.zeros((64, 8), np.float32)
    for c in range(64):
        s[c, c // 8] = 1.0
    return s


def build_kernel(dbg=None):
    nc = bacc.Bacc("TRN2", target_bir_lowering=False, debug=False, num_devices=8)

    x_ext = nc.dram_tensor("x", [C, H, W], F32, kind="ExternalInput").ap()
    rwT_ext = nc.dram_tensor("rwT", [C, CRED], F16, kind="ExternalInput").ap()
    w1T_ext = nc.dram_tensor("w1T", [64, 16], F32, kind="ExternalInput").ap()
    b1_ext = nc.dram_tensor("b1", [16, 1], F32, kind="ExternalInput").ap()
    w2T_ext = nc.dram_tensor("w2T", [16, 64], F32, kind="ExternalInput").ap()
    b2_ext = nc.dram_tensor("b2", [64, 1], F32, kind="ExternalInput").ap()
    gns_ext = nc.dram_tensor("gns", [128, 1], F32, kind="ExternalInput").ap()
    gnb_ext = nc.dram_tensor("gnb", [128, 1], F32, kind="ExternalInput").ap()
    fw1_ext = nc.dram_tensor("fw1", [64, 1], F32, kind="ExternalInput").ap()
    fw2_ext = nc.dram_tensor("fw2", [64, 1], F32, kind="ExternalInput").ap()
    out_ext = nc.dram_tensor("out", [C, H, W], F32, kind="ExternalOutput").ap()

    selg = nc.inline_tensor(_selg_const(), "selg").ap()
    sel8 = nc.inline_tensor(_sel8_const(), "sel8").ap()
    tenth_c = nc.inline_tensor(np.full((1, 128), 0.1, np.float16), "tenth").ap()
    ones_c = nc.inline_tensor(np.ones((1, 128), np.float16), "ones128").ap()
    onesrow_c = nc.inline_tensor(np.ones((1, 320), np.float16), "onesrow").ap()
    ones64_c = nc.inline_tensor(np.ones((64, 1), np.float16), "ones64").ap()

    dbg_ext = None
    if dbg is not None:
        dbg_ext = nc.dram_tensor("dbg", dbg["shape"],
                                 mybir.dt.float16 if dbg.get("f16") else F32,
                                 kind="ExternalOutput").ap()
    with tile.TileContext(nc) as tc:
        _body(tc, nc, x_ext, rwT_ext, w1T_ext, b1_ext, w2T_ext, b2_ext,
              gns_ext, gnb_ext, fw1_ext, fw2_ext, out_ext,
              selg, sel8, tenth_c, ones_c, onesrow_c, ones64_c,
              dbg=dbg, dbg_ext=dbg_ext)

    nc.compile()
    return nc


def _body(tc, nc, x_ext, rwT_ext, w1T_ext, b1_ext, w2T_ext, b2_ext,
          gns_ext, gnb_ext, fw1_ext, fw2_ext, out_ext,
          selg, sel8, tenth_c, ones_c, onesrow_c, ones64_c,
          dbg=None, dbg_ext=None):
    ts = mybir.AluOpType
    AF = mybir.ActivationFunctionType

    # [c, hb, h, w] strided views of the DRAM image tensors
    xv = x_ext.rearrange("c (hb r) w -> c hb r w", hb=2)
    ov = out_ext.rearrange("c (hb r) w -> c hb r w", hb=2)

    from contextlib import ExitStack
    ctx = ExitStack()
    with ctx:
        persist = ctx.enter_context(tc.tile_pool(name="persist", bufs=1))
        # Persistent y field: [128 part, hl 0..81, w 0..161], fp16, row-major.
        Y = persist.tile([128, HL, WP], F16)

        # --- weights / small constants to SBUF ---
        wT0 = persist.tile([128, CRED], F16, tag="wT0")
        wT1 = persist.tile([128, CRED], F16, tag="wT1")
        nc.sync.dma_start(wT0[:], rwT_ext[0:128, :])
        nc.sync.dma_start(wT1[:], rwT_ext[128:256, :])
        selg_sb = persist.tile([64, 8], F32, tag="selg")
        nc.sync.dma_start(selg_sb[:], selg[:])
        sel8_sb = persist.tile([8, 128], F32, tag="sel8")
        nc.sync.dma_start(sel8_sb[:], sel8[:])
        tenth_sb = persist.tile([1, 128], F16, tag="tenth")
        nc.sync.dma_start(tenth_sb[:], tenth_c[:])
        ones_sb = persist.tile([1, 128], F16, tag="ones")
        nc.sync.dma_start(ones_sb[:], ones_c[:])
        onesrow_sb = persist.tile([1, 320], F16, tag="onesrow")
        nc.sync.dma_start(onesrow_sb[:], onesrow_c[:])
        ones64_sb = persist.tile([64, 1], F16, tag="ones64")
        nc.sync.dma_start(ones64_sb[:], ones64_c[:])
        w1T_sb = persist.tile([64, 16], F32, tag="w1T")
        nc.sync.dma_start(w1T_sb[:], w1T_ext[:])
        b1_sb = persist.tile([16, 1], F32, tag="b1")
        nc.sync.dma_start(b1_sb[:], b1_ext[:])
        w2T_sb = persist.tile([16, 64], F32, tag="w2T")
        nc.sync.dma_start(w2T_sb[:], w2T_ext[:])
        b2_sb = persist.tile([64, 1], F32, tag="b2")
        nc.sync.dma_start(b2_sb[:], b2_ext[:])
        gns_sb = persist.tile([128, 1], F32, tag="gns")
        nc.sync.dma_start(gns_sb[:], gns_ext[:])
        gnb_sb = persist.tile([128, 1], F32, tag="gnb")
        nc.sync.dma_start(gnb_sb[:], gnb_ext[:])
        fw1_sb = persist.tile([64, 1], F32, tag="fw1")
        nc.sync.dma_start(fw1_sb[:], fw1_ext[:])
        fw2_sb = persist.tile([64, 1], F32, tag="fw2")
        nc.sync.dma_start(fw2_sb[:], fw2_ext[:])

        # per-channel stat accumulators (both halves on partitions 0..63)
        sacc = persist.tile([64, 2 * NAIT], F32, tag="sacc")
        qacc = persist.tile([64, 2 * NAIT], F32, tag="qacc")

        # C-phase I/O pools opened before phase A (disjoint SBUF, allows
        # C x-prefetch to overlap phase A)
        xc = ctx.enter_context(tc.tile_pool(name="xc", bufs=3))
        oc = ctx.enter_context(tc.tile_pool(name="oc", bufs=3))

        # ---------------- Phase A: y = Wr @ x, stats ----------------
        with tc.tile_pool(name="xa", bufs=6) as xpool, \
             tc.tile_pool(name="cf", bufs=10) as cfpool, \
             tc.tile_pool(name="py", bufs=2, space="PSUM") as pypool, \
             tc.tile_pool(name="stg", bufs=4) as stgpool, \
             tc.tile_pool(name="sq", bufs=4) as sqpool:
            for j in range(NAIT):
                h0 = AH * j
                hl = AH * j + 1
                # ch-half 0: casting DMA (gpsimd); ch-half 1: f32 DMA + DVE cast
                c0 = cfpool.tile([128, 2, AH, W], F16, tag="cf")
                nc.gpsimd.dma_start(c0[:], xv[0:128, :, h0:h0 + AH, :])
                x1 = xpool.tile([128, 2, AH, W], F32, tag="xa")
                nc.sync.dma_start(x1[:], xv[128:256, :, h0:h0 + AH, :])
                c1f = cfpool.tile([128, 2, AH, W], F16, tag="cf")
                nc.vector.tensor_copy(c1f[:], x1[:])
                # psum tiles: [64, rp, 512-padded-bank], rp = row-pair
                pya = pypool.tile([64, 2, 512], F32, tag="pya")
                pyb = pypool.tile([64, 2, 512], F32, tag="pyb")
                for rp in range(2):
                    r0 = 2 * rp
                    nc.tensor.matmul(pya[:, rp, 0:320],
                                     wT0[:], c0[:, 0, r0:r0 + 2, :].rearrange("p r w -> p (r w)"),
                                     start=True, stop=False)
                    nc.tensor.matmul(pya[:, rp, 0:320],
                                     wT1[:], c1f[:, 0, r0:r0 + 2, :].rearrange("p r w -> p (r w)"),
                                     start=False, stop=True)
                    nc.tensor.matmul(pyb[:, rp, 0:320],
                                     wT0[:], c0[:, 1, r0:r0 + 2, :].rearrange("p r w -> p (r w)"),
                                     start=True, stop=False)
                    nc.tensor.matmul(pyb[:, rp, 0:320],
                                     wT1[:], c1f[:, 1, r0:r0 + 2, :].rearrange("p r w -> p (r w)"),
                                     start=False, stop=True)
                pav = pya[:, :, 0:320].rearrange("p a (r w) -> p a r w", r=2)
                pbv = pyb[:, :, 0:320].rearrange("p a (r w) -> p a r w", r=2)
                ydst = Y[0:64, hl:hl + AH, 1:161].rearrange("p (a r) w -> p a r w", a=2)
                # hb0: copy psum -> Y rows directly (+ sum accum) on DVE
                nc.vector.tensor_scalar(ydst, pav, 1.0, 0.0, ts.mult, ts.add,
                                        accum_out=sacc[:, j:j + 1])
                # hb1: copy psum -> staging (+ accum), then DMA into Y[64:128]
                stg = stgpool.tile([64, AH, W], F16, tag="stg")
                sv = stg[:].rearrange("p (a r) w -> p a r w", a=2)
                nc.vector.tensor_scalar(sv, pbv, 1.0, 0.0, ts.mult, ts.add,
                                        accum_out=sacc[:, NAIT + j:NAIT + j + 1])
                nc.sync.dma_start(Y[64:128, hl:hl + AH, 1:161], stg[:])
                # sum of squares (ACT from PSUM)
                sqa = sqpool.tile([64, 2, 320], F16, tag="sqa")
                nc.scalar.activation(sqa[:], pya[:, :, 0:320], AF.Square,
                                     accum_out=qacc[:, j:j + 1])
                sqb = sqpool.tile([64, 2, 320], F16, tag="sqb")
                nc.scalar.activation(sqb[:], pyb[:, :, 0:320], AF.Square,
                                     accum_out=qacc[:, NAIT + j:NAIT + j + 1])

        # halo rows (raw y): global row 80 -> hb0 hl81 ; global row 79 -> hb1 hl0
        nc.sync.dma_start(Y[0:64, 81:82, :], Y[64:128, 1:2, :])
        nc.sync.dma_start(Y[64:128, 0:1, :], Y[0:64, 80:81, :])

        # ---------------- stats + gate (tiny) ----------------
        with tc.tile_pool(name="stat", bufs=1) as stat, \
             tc.tile_pool(name="statp", bufs=1, space="PSUM") as statp:
            SQ = stat.tile([64, 2], F32, tag="SQ")
            nc.vector.tensor_reduce(SQ[:, 0:1], sacc[:], mybir.AxisListType.X, ts.add)
            nc.vector.tensor_reduce(SQ[:, 1:2], qacc[:], mybir.AxisListType.X, ts.add)
            ps8 = statp.tile([8, 2], F32, tag="ps8")
            nc.tensor.matmul(ps8[:], selg_sb[:], SQ[:], start=True, stop=True)

            mi = stat.tile([8, 2], F32, tag="mi")  # col0 mean, col1 invstd
            vtmp = stat.tile([8, 1], F32, tag="vtmp")
            npix_g = float(8 * NPIX)
            nc.vector.tensor_scalar(mi[:, 0:1], ps8[:, 0:1], 1.0 / npix_g, None,
                                    ts.mult)
            nc.vector.tensor_scalar(vtmp[:], ps8[:, 1:2], 1.0 / npix_g, None,
                                    ts.mult)
            msq = stat.tile([8, 1], F32, tag="msq")
            nc.vector.scalar_tensor_tensor(msq[:], mi[:, 0:1], 1.0, mi[:, 0:1],
                                           ts.bypass, ts.mult)
            nc.vector.tensor_sub(vtmp[:], vtmp[:], msq[:])
            nc.vector.tensor_scalar(vtmp[:], vtmp[:], GN_EPS, None, ts.add)
            nc.scalar.activation(vtmp[:], vtmp[:], AF.Sqrt)
            nc.vector.reciprocal(mi[:, 1:2], vtmp[:])
            mi128 = statp.tile([128, 2], F32, tag="mi128")
            nc.tensor.matmul(mi128[:], sel8_sb[:], mi[:], start=True, stop=True)

            # per-partition affine: A = invstd*scale ; Bc = bias - mean*A
            Acoef = persist.tile([128, 1], F32, tag="Acoef")
            Bcoef = persist.tile([128, 1], F32, tag="Bcoef")
            nc.vector.tensor_mul(Acoef[:], mi128[:, 1:2], gns_sb[:])
            tmpB = stat.tile([128, 1], F32, tag="tmpB")
            nc.vector.tensor_mul(tmpB[:], mi128[:, 0:1], Acoef[:])
            nc.vector.tensor_sub(Bcoef[:], gnb_sb[:], tmpB[:])

            # SE gate: p = mean(normalized y) per channel
            pgap = stat.tile([64, 1], F32, tag="pgap")
            nc.vector.tensor_scalar(pgap[:], SQ[:, 0:1], 1.0 / NPIX, None, ts.mult)
            nc.vector.scalar_tensor_tensor(pgap[:], pgap[:], Acoef[0:64, 0:1],
                                           Bcoef[0:64, 0:1], ts.mult, ts.add)
            hdn_ps = statp.tile([16, 1], F32, tag="hdn")
            nc.tensor.matmul(hdn_ps[:], w1T_sb[:], pgap[:], start=True, stop=True)
            hdn = stat.tile([16, 1], F32, tag="hdns")
            nc.scalar.activation(hdn[:], hdn_ps[:], AF.Relu, bias=b1_sb[:, 0:1])
            gam_ps = statp.tile([64, 1], F32, tag="gam")
            nc.tensor.matmul(gam_ps[:], w2T_sb[:], hdn[:], start=True, stop=True)
            gam = stat.tile([64, 1], F32, tag="gams")
            nc.scalar.activation(gam[:], gam_ps[:], AF.Sigmoid, bias=b2_sb[:, 0:1])
            # wc = fw1 + gamma*fw2  (fp16, both partition halves)
            wcf = stat.tile([64, 1], F32, tag="wcf")
            nc.vector.tensor_mul(wcf[:], gam[:], fw2_sb[:])
            nc.vector.tensor_add(wcf[:], wcf[:], fw1_sb[:])
            wcH = persist.tile([128, 1], F16, tag="wcH")
            nc.vector.tensor_copy(wcH[0:64, :], wcf[:])
            nc.sync.dma_start(wcH[64:128, :], wcH[0:64, :])
            wsum_ps = statp.tile([1, 1], F32, tag="wsum_ps")
            nc.tensor.matmul(wsum_ps[:], wcH[0:64, :], ones64_sb[:],
                             start=True, stop=True)
            wsum = persist.tile([1, 1], F32, tag="wsum")
            nc.scalar.copy(wsum[:], wsum_ps[:])

            # B-chain runs on RAW y; normalization folds into ACT abs scales.
            # Pads must hold -B/A (the raw-space image of a zero-padded
            # normalized field).
            rA = stat.tile([128, 1], F32, tag="rA")
            nc.vector.reciprocal(rA[:], Acoef[:])
            pb = persist.tile([128, 1], F32, tag="pb")
            nc.vector.tensor_mul(pb[:], Bcoef[:], rA[:])
            nc.vector.tensor_scalar(pb[:], pb[:], -1.0, None, ts.mult)
            As05 = persist.tile([128, 1], F32, tag="As05")
            nc.vector.tensor_scalar(As05[:], Acoef[:], 0.5, None, ts.mult)
            As225 = persist.tile([128, 1], F32, tag="As225")
            nc.vector.tensor_scalar(As225[:], Acoef[:], 2.25, None, ts.mult)
            for pad, pbs in ((Y[:, :, 0:1], pb[:, 0:1]),
                             (Y[:, :, 161:162], pb[:, 0:1]),
                             (Y[0:64, 0:1, :], pb[0:64, 0:1]),
                             (Y[64:128, 81:82, :], pb[64:128, 0:1])):
                nc.gpsimd.memset(pad, 0.0)
                nc.vector.tensor_single_scalar(pad, pad, pbs, ts.add)
        if dbg and dbg.get("what") == "Y":
            nc.sync.dma_start(dbg_ext[:], Y[:])

        # ---------------- Phase B + C (pipelined over row chunks) ----------------
        with tc.tile_pool(name="bt", bufs=1) as bt, \
             tc.tile_pool(name="tchunk", bufs=4) as tpool, \
             tc.tile_pool(name="gsg", bufs=8) as gsg, \
             tc.tile_pool(name="sps", bufs=4, space="PSUM") as sps, \
             tc.tile_pool(name="gps", bufs=4, space="PSUM") as gps:
            for k in range(NCHUNK):
                a = 1 + CH * k
                c1a = bt.tile([128, CH, WP], F16, tag="c1a")
                nc.vector.tensor_add(c1a[:], Y[:, a - 1:a + CH - 1, :], Y[:, a:a + CH, :])
                c1 = bt.tile([128, CH, WP], F16, tag="c1")
                nc.vector.tensor_add(c1[:], c1a[:], Y[:, a + 1:a + CH + 1, :])
                dv = bt.tile([128, CH, WP], F16, tag="dv")
                nc.gpsimd.tensor_sub(dv[:], Y[:, a - 1:a + CH - 1, :], Y[:, a + 1:a + CH + 1, :])
                av = bt.tile([128, CH, WP], F16, tag="av")
                nc.vector.tensor_add(av[:], c1[:], Y[:, a:a + CH, :])
                ma = bt.tile([128, CH, WP], F16, tag="ma")
                nc.vector.tensor_add(ma[:, :, 0:161], c1[:, :, 0:161], c1[:, :, 1:162])
                m9 = bt.tile([128, CH, W], F16, tag="m9")
                nc.vector.tensor_add(m9[:], ma[:, :, 0:160], c1[:, :, 2:162])
                gx4 = bt.tile([128, CH, W], F16, tag="gx4")
                nc.vector.tensor_sub(gx4[:], av[:, :, 0:160], av[:, :, 2:162])
                e1 = bt.tile([128, CH, WP], F16, tag="e1")
                nc.gpsimd.tensor_add(e1[:, :, 0:161], dv[:, :, 0:161], dv[:, :, 1:162])
                gy4 = bt.tile([128, CH, W], F16, tag="gy4")
                nc.vector.tensor_add(gy4[:], e1[:, :, 0:160], e1[:, :, 1:161])
                y9 = bt.tile([128, CH, W], F16, tag="y9")
                nc.vector.tensor_scalar(y9[:], Y[:, a:a + CH, 1:161], 9.0, None,
                                        ts.mult)
                d9 = bt.tile([128, CH, W], F16, tag="d9")
                nc.vector.tensor_sub(d9[:], y9[:], m9[:])
                # n9h = 0.5*|9y - m|
                n9h = bt.tile([128, CH, W], F16, tag="n9h")
                nc.scalar.activation(n9h[:], d9[:], AF.Abs, scale=As05[:, 0:1])
                ax2 = bt.tile([128, CH, W], F16, tag="ax2")
                nc.scalar.activation(ax2[:], gx4[:], AF.Abs, scale=As225[:, 0:1])
                ay2 = bt.tile([128, CH, W], F16, tag="ay2")
                nc.scalar.activation(ay2[:], gy4[:], AF.Abs, scale=As225[:, 0:1])
                q9 = bt.tile([128, CH, W], F16, tag="q9")
                nc.vector.tensor_add(q9[:], ax2[:], ay2[:])
                Ef = bt.tile([128, CH, W], F32, tag="Ef")
                nc.vector.scalar_tensor_tensor(Ef[:], q9[:], 9.0 * EPS, n9h[:],
                                               ts.add, ts.max)
                rE = bt.tile([128, CH, W], F32, tag="rE")
                nc.vector.reciprocal_approx_fast(rE[:], Ef[:])
                # t/2 = 0.5 * min(ratio, 2); the missing 2x is folded into the
                # sigmoid scale below.
                tch = tpool.tile([128, CH, W], F16, tag="tch")
                nc.vector.tensor_mul(tch[:], n9h[:], rE[:])
                if dbg and dbg.get("what") == "t":
                    nc.sync.dma_start(dbg_ext[:, CH * k:CH * (k + 1), :], tch[:])

                # ---- phase C for this chunk (half-chunks of 4 and 6 rows) ----
                for hh, (hoff, HC) in enumerate(((0, 4), (4, 6))):
                    hc = CH * k + hoff
                    xt0 = xc.tile([128, 2, HC, W], F32, tag="xc")
                    nc.sync.dma_start(xt0[:], xv[0:128, :, hc:hc + HC, :])
                    xt1 = xc.tile([128, 2, HC, W], F32, tag="xc")
                    nc.sync.dma_start(xt1[:], xv[128:256, :, hc:hc + HC, :])
                    ot0 = oc.tile([128, 2, HC, W], F32, tag="oc")
                    ot1 = oc.tile([128, 2, HC, W], F32, tag="oc")
                    sbase = hoff
                    for s2 in range(HC // 2):
                        r2loc = 2 * s2
                        r2 = sbase + r2loc
                        s0 = sps.tile([1, 320], F32, tag="sfuse")
                        s1 = sps.tile([1, 320], F32, tag="sfuse")
                        r0 = tch[0:64, r2:r2 + 2, :].rearrange("p r w -> p (r w)")
                        r1 = tch[64:128, r2:r2 + 2, :].rearrange("p r w -> p (r w)")
                        nc.tensor.matmul(s0[:], wcH[0:64, :], r0, start=True, stop=True)
                        nc.tensor.matmul(s1[:], wcH[64:128, :], r1, start=True, stop=True)
                        g0 = gsg.tile([1, 320], F16, tag="gsig")
                        g1 = gsg.tile([1, 320], F16, tag="gsig")
                        # a = sigmoid(sum(wc) - 2*sum(wc * t/2))
                        nc.scalar.activation(g0[:], s0[:], AF.Sigmoid,
                                             bias=wsum[0:1, 0:1], scale=-2.0)
                        nc.scalar.activation(g1[:], s1[:], AF.Sigmoid,
                                             bias=wsum[0:1, 0:1], scale=-2.0)
                        G0 = gps.tile([128, 320], F32, tag="G")
                        G1 = gps.tile([128, 320], F32, tag="G")
                        nc.tensor.matmul(G0[:], tenth_sb[:], g0[:], start=True, stop=False)
                        nc.tensor.matmul(G0[:], ones_sb[:], onesrow_sb[:], start=False, stop=True)
                        nc.tensor.matmul(G1[:], tenth_sb[:], g1[:], start=True, stop=False)
                        nc.tensor.matmul(G1[:], ones_sb[:], onesrow_sb[:], start=False, stop=True)
                        G1s = gsg.tile([128, 320], F32, tag="G1s")
                        nc.scalar.copy(G1s[:], G1[:])
                        for (xt, ot) in ((xt0, ot0), (xt1, ot1)):
                            nc.vector.tensor_mul(
                                ot[:, 0, r2loc:r2loc + 2, :].rearrange("p r w -> p (r w)"),
                                xt[:, 0, r2loc:r2loc + 2, :].rearrange("p r w -> p (r w)"), G0[:])
                            nc.gpsimd.tensor_mul(
                                ot[:, 1, r2loc:r2loc + 2, :].rearrange("p r w -> p (r w)"),
                                xt[:, 1, r2loc:r2loc + 2, :].rearrange("p r w -> p (r w)"), G1s[:])
                    nc.scalar.dma_start(ov[0:128, :, hc:hc + HC, :], ot0[:])
                    nc.scalar.dma_start(ov[128:256, :, hc:hc + HC, :], ot1[:])


_NC_CACHE = {}


def _get_nc():
    if "nc" not in _NC_CACHE:
        _NC_CACHE["nc"] = build_kernel()
    return _NC_CACHE["nc"]


def kernel(x, reduce_w, gn_scale, gn_bias, gate_w1, gate_b1, gate_w2, gate_b2,
           fuse_w):
    x = np.ascontiguousarray(np.asarray(x, np.float32))
    rwT = np.ascontiguousarray(
        np.asarray(reduce_w, np.float32)[:, :, 0, 0].T.astype(np.float16))
    w1T = np.ascontiguousarray(np.asarray(gate_w1, np.float32)[:, :, 0, 0].T)
    w2T = np.ascontiguousarray(np.asarray(gate_w2, np.float32)[:, :, 0, 0].T)
    b1 = np.asarray(gate_b1, np.float32).reshape(16, 1)
    b2 = np.asarray(gate_b2, np.float32).reshape(64, 1)
    gns = np.ascontiguousarray(np.tile(np.asarray(gn_scale, np.float32), 2).reshape(128, 1))
    gnb = np.ascontiguousarray(np.tile(np.asarray(gn_bias, np.float32), 2).reshape(128, 1))
    fw = np.asarray(fuse_w, np.float32)[0, :, 0, 0]
    fw1 = np.ascontiguousarray(fw[:CRED].reshape(64, 1))
    fw2 = np.ascontiguousarray(fw[CRED:].reshape(64, 1))

    nc = _get_nc()
    shared = dict(rwT=rwT, w1T=w1T, b1=b1, w2T=w2T, b2=b2, gns=gns, gnb=gnb,
                  fw1=fw1, fw2=fw2)
    in_maps = [dict(x=np.ascontiguousarray(x[i]), **shared) for i in range(B)]
    res = run_bass_kernel_spmd(nc, in_maps, core_ids=list(range(8)))
    return np.stack([res.results[i]["out"] for i in range(B)], axis=0)


# revision 26
# speedup vs baseline: 1.1088x; 1.1088x over previous
"""Trainium2 Bass kernel for nn_ADRC_PE (dense CNN: 1x1 reduce -> GroupNorm ->
fixed 3x3 depthwise convs -> curvature gate -> fuse -> residual scale).

Sharding: pure data parallel, batch dim (B=8) across 8 NeuronCores.
Per-core field layout: 128 partitions = 64 channels x 2 row-halves
(partition c holds image rows 0..79 of channel c, partition 64+c rows
80..159), fp16, row-major [hl, w] with 1-row halos and zero padding.
"""

import numpy as np

import concourse.bass as bass
import concourse.tile as tile
from concourse import bacc, mybir
from concourse.bass_utils import run_bass_kernel_spmd

F32 = mybir.dt.float32
F16 = mybir.dt.float16

B, C, H, W = 8, 256, 160, 160
CRED, GROUPS = 64, 8
EPS, GN_EPS = 1e-4, 1e-5

HL = 82          # local h extent per half-block (1 halo/pad row each side)
WP = 162         # padded w extent
NPIX = H * W     # 25600
CH = 10          # phase-B chunk: output rows per chunk (per half-block)
NCHUNK = 80 // CH
AH = 4           # phase-A rows per half-block per iteration
NAIT = 80 // AH  # 20


def _sel8_const():
    """[8, 128] fp32: sel8[g, p] = 1 if channel-group of p == g (broadcast)."""
    s = np.zeros((8, 128), np.float32)
    for p in range(128):
        s[(p % 64) // 8, p] = 1.0
    return s


def _selg_const():
    """[64, 8] fp32: selg[c, g] = 1 if group of channel c == g (group sums)."""
    s = np.zeros((64, 8), np.float32)
    for c in range(64):
        s[c, c // 8] = 1.0
    return s


def build_kernel(dbg=None):
    nc = bacc.Bacc("TRN2", target_bir_lowering=False, debug=False, num_devices=8)

    x_ext = nc.dram_tensor("x", [C, H, W], F32, kind="ExternalInput").ap()
    rwT_ext = nc.dram_tensor("rwT", [C, CRED], F16, kind="ExternalInput").ap()
    w1T_ext = nc.dram_tensor("w1T", [64, 16], F32, kind="ExternalInput").ap()
    b1_ext = nc.dram_tensor("b1", [16, 1], F32, kind="ExternalInput").ap()
    w2T_ext = nc.dram_tensor("w2T", [16, 64], F32, kind="ExternalInput").ap()
    b2_ext = nc.dram_tensor("b2", [64, 1], F32, kind="ExternalInput").ap()
    gns_ext = nc.dram_tensor("gns", [128, 1], F32, kind="ExternalInput").ap()
    gnb_ext = nc.dram_tensor("gnb", [128, 1], F32, kind="ExternalInput").ap()
    fw1_ext = nc.dram_tensor("fw1", [64, 1], F32, kind="ExternalInput").ap()
    fw2_ext = nc.dram_tensor("fw2", [64, 1], F32, kind="ExternalInput").ap()
    out_ext = nc.dram_tensor("out", [C, H, W], F32, kind="ExternalOutput").ap()

    selg = nc.inline_tensor(_selg_const(), "selg").ap()
    sel8 = nc.inline_tensor(_sel8_const(), "sel8").ap()
    tenth_c = nc.inline_tensor(np.full((1, 128), 0.1, np.float16), "tenth").ap()
    ones_c = nc.inline_tensor(np.ones((1, 128), np.float16), "ones128").ap()
    onesrow_c = nc.inline_tensor(np.ones((1, 320), np.float16), "onesrow").ap()
    ones64_c = nc.inline_tensor(np.ones((64, 1), np.float16), "ones64").ap()

    dbg_ext = None
    if dbg is not None:
        dbg_ext = nc.dram_tensor("dbg", dbg["shape"],
                                 mybir.dt.float16 if dbg.get("f16") else F32,
                                 kind="ExternalOutput").ap()
    with tile.TileContext(nc) as tc:
        _body(tc, nc, x_ext, rwT_ext, w1T_ext, b1_ext, w2T_ext, b2_ext,
              gns_ext, gnb_ext, fw1_ext, fw2_ext, out_ext,
              selg, sel8, tenth_c, ones_c, onesrow_c, ones64_c,
              dbg=dbg, dbg_ext=dbg_ext)

    nc.compile()
    return nc


def _body(tc, nc, x_ext, rwT_ext, w1T_ext, b1_ext, w2T_ext, b2_ext,
          gns_ext, gnb_ext, fw1_ext, fw2_ext, out_ext,
          selg, sel8, tenth_c, ones_c, onesrow_c, ones64_c,
          dbg=None, dbg_ext=None):
    ts = mybir.AluOpType
    AF = mybir.ActivationFunctionType

    # [c, hb, h, w] strided views of the DRAM image tensors
    xv = x_ext.rearrange("c (hb r) w -> c hb r w", hb=2)
    ov = out_ext.rearrange("c (hb r) w -> c hb r w", hb=2)

    from contextlib import ExitStack
    ctx = ExitStack()
    with ctx:
        persist = ctx.enter_context(tc.tile_pool(name="persist", bufs=1))
        # Persistent y field: [128 part, hl 0..81, w 0..161], fp16, row-major.
        Y = persist.tile([128, HL, WP], F16)

        # --- weights / small constants to SBUF ---
        wT0 = persist.tile([128, CRED], F16, tag="wT0")
        wT1 = persist.tile([128, CRED], F16, tag="wT1")
        nc.sync.dma_start(wT0[:], rwT_ext[0:128, :])
        nc.sync.dma_start(wT1[:], rwT_ext[128:256, :])
        selg_sb = persist.tile([64, 8], F32, tag="selg")
        nc.sync.dma_start(selg_sb[:], selg[:])
        sel8_sb = persist.tile([8, 128], F32, tag="sel8")
        nc.sync.dma_start(sel8_sb[:], sel8[:])
        tenth_sb = persist.tile([1, 128], F16, tag="tenth")
        nc.sync.dma_start(tenth_sb[:], tenth_c[:])
        ones_sb = persist.tile([1, 128], F16, tag="ones")
        nc.sync.dma_start(ones_sb[:], ones_c[:])
        onesrow_sb = persist.tile([1, 320], F16, tag="onesrow")
        nc.sync.dma_start(onesrow_sb[:], onesrow_c[:])
        ones64_sb = persist.tile([64, 1], F16, tag="ones64")
        nc.sync.dma_start(ones64_sb[:], ones64_c[:])
        w1T_sb = persist.tile([64, 16], F32, tag="w1T")
        nc.sync.dma_start(w1T_sb[:], w1T_ext[:])
        b1_sb = persist.tile([16, 1], F32, tag="b1")
        nc.sync.dma_start(b1_sb[:], b1_ext[:])
        w2T_sb = persist.tile([16, 64], F32, tag="w2T")
        nc.sync.dma_start(w2T_sb[:], w2T_ext[:])
        b2_sb = persist.tile([64, 1], F32, tag="b2")
        nc.sync.dma_start(b2_sb[:], b2_ext[:])
        gns_sb = persist.tile([128, 1], F32, tag="gns")
        nc.sync.dma_start(gns_sb[:], gns_ext[:])
        gnb_sb = persist.tile([128, 1], F32, tag="gnb")
        nc.sync.dma_start(gnb_sb[:], gnb_ext[:])
        fw1_sb = persist.tile([64, 1], F32, tag="fw1")
        nc.sync.dma_start(fw1_sb[:], fw1_ext[:])
        fw2_sb = persist.tile([64, 1], F32, tag="fw2")
        nc.sync.dma_start(fw2_sb[:], fw2_ext[:])

        # per-channel stat accumulators (both halves on partitions 0..63)
        sacc = persist.tile([64, 2 * NAIT], F32, tag="sacc")
        qacc = persist.tile([64, 2 * NAIT], F32, tag="qacc")

        # C-phase I/O pools opened before phase A (disjoint SBUF, allows
        # C x-prefetch to overlap phase A)
        xc = ctx.enter_context(tc.tile_pool(name="xc", bufs=3))
        oc = ctx.enter_context(tc.tile_pool(name="oc", bufs=3))

        # ---------------- Phase A: y = Wr @ x, stats ----------------
        with tc.tile_pool(name="xa", bufs=6) as xpool, \
             tc.tile_pool(name="cf", bufs=10) as cfpool, \
             tc.tile_pool(name="py", bufs=2, space="PSUM") as pypool, \
             tc.tile_pool(name="stg", bufs=4) as stgpool, \
             tc.tile_pool(name="sq", bufs=4) as sqpool:
            for j in range(NAIT):
                h0 = AH * j
                hl = AH * j + 1
                # ch-half 0: casting DMA (gpsimd); ch-half 1: f32 DMA + DVE cast
                c0 = cfpool.tile([128, 2, AH, W], F16, tag="cf")
                nc.gpsimd.dma_start(c0[:], xv[0:128, :, h0:h0 + AH, :])
                x1 = xpool.tile([128, 2, AH, W], F32, tag="xa")
                nc.sync.dma_start(x1[:], xv[128:256, :, h0:h0 + AH, :])
                c1f = cfpool.tile([128, 2, AH, W], F16, tag="cf")
                nc.vector.tensor_copy(c1f[:], x1[:])
                # psum tiles: [64, rp, 512-padded-bank], rp = row-pair
                pya = pypool.tile([64, 2, 512], F32, tag="pya")
                pyb = pypool.tile([64, 2, 512], F32, tag="pyb")
                for rp in range(2):
                    r0 = 2 * rp
                    nc.tensor.matmul(pya[:, rp, 0:320],
                                     wT0[:], c0[:, 0, r0:r0 + 2, :].rearrange("p r w -> p (r w)"),
                                     start=True, stop=False)
                    nc.tensor.matmul(pya[:, rp, 0:320],
                                     wT1[:], c1f[:, 0, r0:r0 + 2, :].rearrange("p r w -> p (r w)"),
                                     start=False, stop=True)
                    nc.tensor.matmul(pyb[:, rp, 0:320],
                                     wT0[:], c0[:, 1, r0:r0 + 2, :].rearrange("p r w -> p (r w)"),
                                     start=True, stop=False)
                    nc.tensor.matmul(pyb[:, rp, 0:320],
                                     wT1[:], c1f[:, 1, r0:r0 + 2, :].rearrange("p r w -> p (r w)"),
                                     start=False, stop=True)
                pav = pya[:, :, 0:320].rearrange("p a (r w) -> p a r w", r=2)
                pbv = pyb[:, :, 0:320].rearrange("p a (r w) -> p a r w", r=2)
                ydst = Y[0:64, hl:hl + AH, 1:161].rearrange("p (a r) w -> p a r w", a=2)
                # hb0: copy psum -> Y rows directly (+ sum accum) on DVE
                nc.vector.tensor_scalar(ydst, pav, 1.0, 0.0, ts.mult, ts.add,
                                        accum_out=sacc[:, j:j + 1])
                # hb1: copy psum -> staging (+ accum), then DMA into Y[64:128]
                stg = stgpool.tile([64, AH, W], F16, tag="stg")
                sv = stg[:].rearrange("p (a r) w -> p a r w", a=2)
                nc.vector.tensor_scalar(sv, pbv, 1.0, 0.0, ts.mult, ts.add,
                                        accum_out=sacc[:, NAIT + j:NAIT + j + 1])
                nc.sync.dma_start(Y[64:128, hl:hl + AH, 1:161], stg[:])
                # sum of squares (ACT from PSUM)
                sqa = sqpool.tile([64, 2, 320], F16, tag="sqa")
                nc.scalar.activation(sqa[:], pya[:, :, 0:320], AF.Square,
                                     accum_out=qacc[:, j:j + 1])
                sqb = sqpool.tile([64, 2, 320], F16, tag="sqb")
                nc.scalar.activation(sqb[:], pyb[:, :, 0:320], AF.Square,
                                     accum_out=qacc[:, NAIT + j:NAIT + j + 1])

        # halo rows (raw y): global row 80 -> hb0 hl81 ; global row 79 -> hb1 hl0
        nc.scalar.dma_start(Y[0:64, 81:82, :], Y[64:128, 1:2, :])
        nc.scalar.dma_start(Y[64:128, 0:1, :], Y[0:64, 80:81, :])

        # ---------------- stats + gate (tiny) ----------------
        with tc.tile_pool(name="stat", bufs=1) as stat, \
             tc.tile_pool(name="statp", bufs=1, space="PSUM") as statp:
            SQ = stat.tile([64, 2], F32, tag="SQ")
            nc.vector.tensor_reduce(SQ[:, 0:1], sacc[:], mybir.AxisListType.X, ts.add)
            nc.vector.tensor_reduce(SQ[:, 1:2], qacc[:], mybir.AxisListType.X, ts.add)
            ps8 = statp.tile([8, 2], F32, tag="ps8")
            nc.tensor.matmul(ps8[:], selg_sb[:], SQ[:], start=True, stop=True)

            mi = stat.tile([8, 2], F32, tag="mi")  # col0 mean, col1 invstd
            vtmp = stat.tile([8, 1], F32, tag="vtmp")
            npix_g = float(8 * NPIX)
            nc.vector.tensor_scalar(mi[:, 0:1], ps8[:, 0:1], 1.0 / npix_g, None,
                                    ts.mult)
            nc.vector.tensor_scalar(vtmp[:], ps8[:, 1:2], 1.0 / npix_g, None,
                                    ts.mult)
            msq = stat.tile([8, 1], F32, tag="msq")
            nc.vector.scalar_tensor_tensor(msq[:], mi[:, 0:1], 1.0, mi[:, 0:1],
                                           ts.bypass, ts.mult)
            nc.vector.tensor_sub(vtmp[:], vtmp[:], msq[:])
            nc.vector.tensor_scalar(vtmp[:], vtmp[:], GN_EPS, None, ts.add)
            nc.scalar.activation(vtmp[:], vtmp[:], AF.Sqrt)
            nc.vector.reciprocal(mi[:, 1:2], vtmp[:])
            mi128 = statp.tile([128, 2], F32, tag="mi128")
            nc.tensor.matmul(mi128[:], sel8_sb[:], mi[:], start=True, stop=True)

            # per-partition affine: A = invstd*scale ; Bc = bias - mean*A
            Acoef = persist.tile([128, 1], F32, tag="Acoef")
            Bcoef = persist.tile([128, 1], F32, tag="Bcoef")
            nc.vector.tensor_mul(Acoef[:], mi128[:, 1:2], gns_sb[:])
            tmpB = stat.tile([128, 1], F32, tag="tmpB")
            nc.vector.tensor_mul(tmpB[:], mi128[:, 0:1], Acoef[:])
            nc.vector.tensor_sub(Bcoef[:], gnb_sb[:], tmpB[:])

            # SE gate: p = mean(normalized y) per channel
            pgap = stat.tile([64, 1], F32, tag="pgap")
            nc.vector.tensor_scalar(pgap[:], SQ[:, 0:1], 1.0 / NPIX, None, ts.mult)
            nc.vector.scalar_tensor_tensor(pgap[:], pgap[:], Acoef[0:64, 0:1],
                                           Bcoef[0:64, 0:1], ts.mult, ts.add)
            hdn_ps = statp.tile([16, 1], F32, tag="hdn")
            nc.tensor.matmul(hdn_ps[:], w1T_sb[:], pgap[:], start=True, stop=True)
            hdn = stat.tile([16, 1], F32, tag="hdns")
            nc.scalar.activation(hdn[:], hdn_ps[:], AF.Relu, bias=b1_sb[:, 0:1])
            gam_ps = statp.tile([64, 1], F32, tag="gam")
            nc.tensor.matmul(gam_ps[:], w2T_sb[:], hdn[:], start=True, stop=True)
            gam = stat.tile([64, 1], F32, tag="gams")
            nc.scalar.activation(gam[:], gam_ps[:], AF.Sigmoid, bias=b2_sb[:, 0:1])
            # wc = fw1 + gamma*fw2  (fp16, both partition halves)
            wcf = stat.tile([64, 1], F32, tag="wcf")
            nc.vector.tensor_mul(wcf[:], gam[:], fw2_sb[:])
            nc.vector.tensor_add(wcf[:], wcf[:], fw1_sb[:])
            wcH = persist.tile([128, 1], F16, tag="wcH")
            nc.vector.tensor_copy(wcH[0:64, :], wcf[:])
            nc.scalar.dma_start(wcH[64:128, :], wcH[0:64, :])
            wsum_ps = statp.tile([1, 1], F32, tag="wsum_ps")
            nc.tensor.matmul(wsum_ps[:], wcH[0:64, :], ones64_sb[:],
                             start=True, stop=True)
            wsum = persist.tile([1, 1], F32, tag="wsum")
            nc.scalar.copy(wsum[:], wsum_ps[:])

            # B-chain runs on RAW y; normalization folds into ACT abs scales.
            # Pads must hold -B/A (the raw-space image of a zero-padded
            # normalized field).
            rA = stat.tile([128, 1], F32, tag="rA")
            nc.vector.reciprocal(rA[:], Acoef[:])
            pb = persist.tile([128, 1], F32, tag="pb")
            nc.vector.tensor_mul(pb[:], Bcoef[:], rA[:])
            nc.vector.tensor_scalar(pb[:], pb[:], -1.0, None, ts.mult)
            As05 = persist.tile([128, 1], F32, tag="As05")
            nc.vector.tensor_scalar(As05[:], Acoef[:], 0.5, None, ts.mult)
            As225 = persist.tile([128, 1], F32, tag="As225")
            nc.vector.tensor_scalar(As225[:], Acoef[:], 2.25, None, ts.mult)
            for pad, pbs in ((Y[:, :, 0:1], pb[:, 0:1]),
                             (Y[:, :, 161:162], pb[:, 0:1]),
                             (Y[0:64, 0:1, :], pb[0:64, 0:1]),
                             (Y[64:128, 81:82, :], pb[64:128, 0:1])):
                nc.gpsimd.memset(pad, 0.0)
                nc.vector.tensor_single_scalar(pad, pad, pbs, ts.add)
        if dbg and dbg.get("what") == "Y":
            nc.sync.dma_start(dbg_ext[:], Y[:])

        # ---------------- Phase B + C (pipelined over row chunks) ----------------
        with tc.tile_pool(name="bt", bufs=1) as bt, \
             tc.tile_pool(name="tchunk", bufs=4) as tpool, \
             tc.tile_pool(name="gsg", bufs=8) as gsg, \
             tc.tile_pool(name="sps", bufs=4, space="PSUM") as sps, \
             tc.tile_pool(name="gps", bufs=4, space="PSUM") as gps:
            for k in range(NCHUNK):
                a = 1 + CH * k
                c1a = bt.tile([128, CH, WP], F16, tag="c1a")
                nc.vector.tensor_add(c1a[:], Y[:, a - 1:a + CH - 1, :], Y[:, a:a + CH, :])
                c1 = bt.tile([128, CH, WP], F16, tag="c1")
                nc.vector.tensor_add(c1[:], c1a[:], Y[:, a + 1:a + CH + 1, :])
                dv = bt.tile([128, CH, WP], F16, tag="dv")
                nc.gpsimd.tensor_sub(dv[:], Y[:, a - 1:a + CH - 1, :], Y[:, a + 1:a + CH + 1, :])
                av = bt.tile([128, CH, WP], F16, tag="av")
                nc.vector.tensor_add(av[:], c1[:], Y[:, a:a + CH, :])
                ma = bt.tile([128, CH, WP], F16, tag="ma")
                nc.vector.tensor_add(ma[:, :, 0:161], c1[:, :, 0:161], c1[:, :, 1:162])
                m9 = bt.tile([128, CH, W], F16, tag="m9")
                nc.vector.tensor_add(m9[:], ma[:, :, 0:160], c1[:, :, 2:162])
                gx4 = bt.tile([128, CH, W], F16, tag="gx4")
                nc.vector.tensor_sub(gx4[:], av[:, :, 0:160], av[:, :, 2:162])
                e1 = bt.tile([128, CH, WP], F16, tag="e1")
                nc.gpsimd.tensor_add(e1[:, :, 0:161], dv[:, :, 0:161], dv[:, :, 1:162])
                gy4 = bt.tile([128, CH, W], F16, tag="gy4")
                nc.vector.tensor_add(gy4[:], e1[:, :, 0:160], e1[:, :, 1:161])
                y9 = bt.tile([128, CH, W], F16, tag="y9")
                nc.vector.tensor_scalar(y9[:], Y[:, a:a + CH, 1:161], 9.0, None,
                                        ts.mult)
                d9 = bt.tile([128, CH, W], F16, tag="d9")
                nc.vector.tensor_sub(d9[:], y9[:], m9[:])
                # n9h = 0.5*|9y - m|
                n9h = bt.tile([128, CH, W], F16, tag="n9h")
                nc.scalar.activation(n9h[:], d9[:], AF.Abs, scale=As05[:, 0:1])
                ax2 = bt.tile([128, CH, W], F16, tag="ax2")
                nc.scalar.activation(ax2[:], gx4[:], AF.Abs, scale=As225[:, 0:1])
                ay2 = bt.tile([128, CH, W], F16, tag="ay2")
                nc.scalar.activation(ay2[:], gy4[:], AF.Abs, scale=As225[:, 0:1])
                q9 = bt.tile([128, CH, W], F16, tag="q9")
                nc.vector.tensor_add(q9[:], ax2[:], ay2[:])
                Ef = bt.tile([128, CH, W], F32, tag="Ef")
                nc.vector.scalar_tensor_tensor(Ef[:], q9[:], 9.0 * EPS, n9h[:],
                                               ts.add, ts.max)
                rE = bt.tile([128, CH, W], F32, tag="rE")
                nc.vector.reciprocal_approx_fast(rE[:], Ef[:])
                # t/2 = 0.5 * min(ratio, 2); the missing 2x is folded into the
                # sigmoid scale below.
                tch = tpool.tile([128, CH, W], F16, tag="tch")
                nc.vector.tensor_mul(tch[:], n9h[:], rE[:])
                if dbg and dbg.get("what") == "t":
                    nc.sync.dma_start(dbg_ext[:, CH * k:CH * (k + 1), :], tch[:])

                # ---- phase C for this chunk (half-chunks of 4 and 6 rows) ----
                for hh, (hoff, HC) in enumerate(((0, 4), (4, 6))):
                    hc = CH * k + hoff
                    xt0 = xc.tile([128, 2, HC, W], F32, tag="xc")
                    nc.sync.dma_start(xt0[:], xv[0:128, :, hc:hc + HC, :])
                    xt1 = xc.tile([128, 2, HC, W], F32, tag="xc")
                    nc.sync.dma_start(xt1[:], xv[128:256, :, hc:hc + HC, :])
                    ot0 = oc.tile([128, 2, HC, W], F32, tag="oc")
                    ot1 = oc.tile([128, 2, HC, W], F32, tag="oc")
                    sbase = hoff
                    for s2 in range(HC // 2):
                        r2loc = 2 * s2
                        r2 = sbase + r2loc
                        s0 = sps.tile([1, 320], F32, tag="sfuse")
                        s1 = sps.tile([1, 320], F32, tag="sfuse")
                        r0 = tch[0:64, r2:r2 + 2, :].rearrange("p r w -> p (r w)")
                        r1 = tch[64:128, r2:r2 + 2, :].rearrange("p r w -> p (r w)")
                        nc.tensor.matmul(s0[:], wcH[0:64, :], r0, start=True, stop=True)
                        nc.tensor.matmul(s1[:], wcH[64:128, :], r1, start=True, stop=True)
                        g0 = gsg.tile([1, 320], F16, tag="gsig")
                        g1 = gsg.tile([1, 320], F16, tag="gsig")
                        # a = sigmoid(sum(wc) - 2*sum(wc * t/2))
                        nc.scalar.activation(g0[:], s0[:], AF.Sigmoid,
                                             bias=wsum[0:1, 0:1], scale=-2.0)
                        nc.scalar.activation(g1[:], s1[:], AF.Sigmoid,
                                             bias=wsum[0:1, 0:1], scale=-2.0)
                        G0 = gps.tile([128, 320], F32, tag="G")
                        G1 = gps.tile([128, 320], F32, tag="G")
                        nc.tensor.matmul(G0[:], tenth_sb[:], g0[:], start=True, stop=False)
                        nc.tensor.matmul(G0[:], ones_sb[:], onesrow_sb[:], start=False, stop=True)
                        nc.tensor.matmul(G1[:], tenth_sb[:], g1[:], start=True, stop=False)
                        nc.tensor.matmul(G1[:], ones_sb[:], onesrow_sb[:], start=False, stop=True)
                        G1s = gsg.tile([128, 320], F32, tag="G1s")
                        nc.scalar.copy(G1s[:], G1[:])
                        for (xt, ot) in ((xt0, ot0), (xt1, ot1)):
                            nc.vector.tensor_mul(
                                ot[:, 0, r2loc:r2loc + 2, :].rearrange("p r w -> p (r w)"),
                                xt[:, 0, r2loc:r2loc + 2, :].rearrange("p r w -> p (r w)"), G0[:])
                            nc.gpsimd.tensor_mul(
                                ot[:, 1, r2loc:r2loc + 2, :].rearrange("p r w -> p (r w)"),
                                xt[:, 1, r2loc:r2loc + 2, :].rearrange("p r w -> p (r w)"), G1s[:])
                    nc.scalar.dma_start(ov[0:128, :, hc:hc + HC, :], ot0[:])
                    nc.scalar.dma_start(ov[128:256, :, hc:hc + HC, :], ot1[:])


_NC_CACHE = {}


def _get_nc():
    if "nc" not in _NC_CACHE:
        _NC_CACHE["nc"] = build_kernel()
    return _NC_CACHE["nc"]


def kernel(x, reduce_w, gn_scale, gn_bias, gate_w1, gate_b1, gate_w2, gate_b2,
           fuse_w):
    x = np.ascontiguousarray(np.asarray(x, np.float32))
    rwT = np.ascontiguousarray(
        np.asarray(reduce_w, np.float32)[:, :, 0, 0].T.astype(np.float16))
    w1T = np.ascontiguousarray(np.asarray(gate_w1, np.float32)[:, :, 0, 0].T)
    w2T = np.ascontiguousarray(np.asarray(gate_w2, np.float32)[:, :, 0, 0].T)
    b1 = np.asarray(gate_b1, np.float32).reshape(16, 1)
    b2 = np.asarray(gate_b2, np.float32).reshape(64, 1)
    gns = np.ascontiguousarray(np.tile(np.asarray(gn_scale, np.float32), 2).reshape(128, 1))
    gnb = np.ascontiguousarray(np.tile(np.asarray(gn_bias, np.float32), 2).reshape(128, 1))
    fw = np.asarray(fuse_w, np.float32)[0, :, 0, 0]
    fw1 = np.ascontiguousarray(fw[:CRED].reshape(64, 1))
    fw2 = np.ascontiguousarray(fw[CRED:].reshape(64, 1))

    nc = _get_nc()
    shared = dict(rwT=rwT, w1T=w1T, b1=b1, w2T=w2T, b2=b2, gns=gns, gnb=gnb,
                  fw1=fw1, fw2=fw2)
    in_maps = [dict(x=np.ascontiguousarray(x[i]), **shared) for i in range(B)]
    res = run_bass_kernel_spmd(nc, in_maps, core_ids=list(range(8)))
    return np.stack([res.results[i]["out"] for i in range(B)], axis=0)


# revision 27
# speedup vs baseline: 1.2400x; 1.1183x over previous
"""Trainium2 Bass kernel for nn_ADRC_PE (dense CNN: 1x1 reduce -> GroupNorm ->
fixed 3x3 depthwise convs -> curvature gate -> fuse -> residual scale).

Sharding: pure data parallel, batch dim (B=8) across 8 NeuronCores.
Per-core field layout: 128 partitions = 64 channels x 2 row-halves
(partition c holds image rows 0..79 of channel c, partition 64+c rows
80..159), fp16, row-major [hl, w] with 1-row halos and zero padding.
"""

import numpy as np

import concourse.bass as bass
import concourse.tile as tile
from concourse import bacc, mybir
from concourse.bass_utils import run_bass_kernel_spmd

F32 = mybir.dt.float32
F16 = mybir.dt.float16

B, C, H, W = 8, 256, 160, 160
CRED, GROUPS = 64, 8
EPS, GN_EPS = 1e-4, 1e-5

HL = 82          # local h extent per half-block (1 halo/pad row each side)
WP = 162         # padded w extent
NPIX = H * W     # 25600
CH = 10          # phase-B chunk: output rows per chunk (per half-block)
NCHUNK = 80 // CH
AH = 4           # phase-A rows per half-block per iteration
NAIT = 80 // AH  # 20


def _sel8_const():
    """[8, 128] fp32: sel8[g, p] = 1 if channel-group of p == g (broadcast)."""
    s = np.zeros((8, 128), np.float32)
    for p in range(128):
        s[(p % 64) // 8, p] = 1.0
    return s


def _selg_const():
    """[64, 8] fp32: selg[c, g] = 1 if group of channel c == g (group sums)."""
    s = np.zeros((64, 8), np.float32)
    for c in range(64):
        s[c, c // 8] = 1.0
    return s


def build_kernel(dbg=None):
    nc = bacc.Bacc("TRN2", target_bir_lowering=False, debug=False, num_devices=8)

    x_ext = nc.dram_tensor("x", [C, H, W], F32, kind="ExternalInput").ap()
    rwT_ext = nc.dram_tensor("rwT", [C, CRED], F16, kind="ExternalInput").ap()
    w1T_ext = nc.dram_tensor("w1T", [64, 16], F32, kind="ExternalInput").ap()
    b1_ext = nc.dram_tensor("b1", [16, 1], F32, kind="ExternalInput").ap()
    w2T_ext = nc.dram_tensor("w2T", [16, 64], F32, kind="ExternalInput").ap()
    b2_ext = nc.dram_tensor("b2", [64, 1], F32, kind="ExternalInput").ap()
    gns_ext = nc.dram_tensor("gns", [128, 1], F32, kind="ExternalInput").ap()
    gnb_ext = nc.dram_tensor("gnb", [128, 1], F32, kind="ExternalInput").ap()
    fw1_ext = nc.dram_tensor("fw1", [64, 1], F32, kind="ExternalInput").ap()
    fw2_ext = nc.dram_tensor("fw2", [64, 1], F32, kind="ExternalInput").ap()
    out_ext = nc.dram_tensor("out", [C, H, W], F32, kind="ExternalOutput").ap()

    selg = nc.inline_tensor(_selg_const(), "selg").ap()
    sel8 = nc.inline_tensor(_sel8_const(), "sel8").ap()
    tenth_c = nc.inline_tensor(np.full((1, 128), 0.1, np.float16), "tenth").ap()
    ones_c = nc.inline_tensor(np.ones((1, 128), np.float16), "ones128").ap()
    onesrow_c = nc.inline_tensor(np.ones((1, 320), np.float16), "onesrow").ap()
    ones64_c = nc.inline_tensor(np.ones((64, 1), np.float16), "ones64").ap()

    dbg_ext = None
    if dbg is not None:
        dbg_ext = nc.dram_tensor("dbg", dbg["shape"],
                                 mybir.dt.float16 if dbg.get("f16") else F32,
                                 kind="ExternalOutput").ap()
    with tile.TileContext(nc) as tc:
        _body(tc, nc, x_ext, rwT_ext, w1T_ext, b1_ext, w2T_ext, b2_ext,
              gns_ext, gnb_ext, fw1_ext, fw2_ext, out_ext,
              selg, sel8, tenth_c, ones_c, onesrow_c, ones64_c,
              dbg=dbg, dbg_ext=dbg_ext)

    nc.compile()
    return nc


def _body(tc, nc, x_ext, rwT_ext, w1T_ext, b1_ext, w2T_ext, b2_ext,
          gns_ext, gnb_ext, fw1_ext, fw2_ext, out_ext,
          selg, sel8, tenth_c, ones_c, onesrow_c, ones64_c,
          dbg=None, dbg_ext=None):
    ts = mybir.AluOpType
    AF = mybir.ActivationFunctionType

    # [c, hb, h, w] strided views of the DRAM image tensors
    xv = x_ext.rearrange("c (hb r) w -> c hb r w", hb=2)
    ov = out_ext.rearrange("c (hb r) w -> c hb r w", hb=2)

    from contextlib import ExitStack
    ctx = ExitStack()
    with ctx:
        persist = ctx.enter_context(tc.tile_pool(name="persist", bufs=1))
        # Persistent y field: [128 part, hl 0..81, w 0..161], fp16, row-major.
        Y = persist.tile([128, HL, WP], F16)

        # --- weights / small constants to SBUF ---
        wT0 = persist.tile([128, CRED], F16, tag="wT0")
        wT1 = persist.tile([128, CRED], F16, tag="wT1")
        nc.sync.dma_start(wT0[:], rwT_ext[0:128, :])
        nc.sync.dma_start(wT1[:], rwT_ext[128:256, :])
        selg_sb = persist.tile([64, 8], F32, tag="selg")
        nc.sync.dma_start(selg_sb[:], selg[:])
        sel8_sb = persist.tile([8, 128], F32, tag="sel8")
        nc.sync.dma_start(sel8_sb[:], sel8[:])
        tenth_sb = persist.tile([1, 128], F16, tag="tenth")
        nc.sync.dma_start(tenth_sb[:], tenth_c[:])
        ones_sb = persist.tile([1, 128], F16, tag="ones")
        nc.sync.dma_start(ones_sb[:], ones_c[:])
        onesrow_sb = persist.tile([1, 320], F16, tag="onesrow")
        nc.sync.dma_start(onesrow_sb[:], onesrow_c[:])
        ones64_sb = persist.tile([64, 1], F16, tag="ones64")
        nc.sync.dma_start(ones64_sb[:], ones64_c[:])
        w1T_sb = persist.tile([64, 16], F32, tag="w1T")
        nc.sync.dma_start(w1T_sb[:], w1T_ext[:])
        b1_sb = persist.tile([16, 1], F32, tag="b1")
        nc.sync.dma_start(b1_sb[:], b1_ext[:])
        w2T_sb = persist.tile([16, 64], F32, tag="w2T")
        nc.sync.dma_start(w2T_sb[:], w2T_ext[:])
        b2_sb = persist.tile([64, 1], F32, tag="b2")
        nc.sync.dma_start(b2_sb[:], b2_ext[:])
        gns_sb = persist.tile([128, 1], F32, tag="gns")
        nc.sync.dma_start(gns_sb[:], gns_ext[:])
        gnb_sb = persist.tile([128, 1], F32, tag="gnb")
        nc.sync.dma_start(gnb_sb[:], gnb_ext[:])
        fw1_sb = persist.tile([64, 1], F32, tag="fw1")
        nc.sync.dma_start(fw1_sb[:], fw1_ext[:])
        fw2_sb = persist.tile([64, 1], F32, tag="fw2")
        nc.sync.dma_start(fw2_sb[:], fw2_ext[:])

        # per-channel stat accumulators (both halves on partitions 0..63)
        sacc = persist.tile([64, 2 * NAIT], F32, tag="sacc")
        qacc = persist.tile([64, 2 * NAIT], F32, tag="qacc")

        # C-phase I/O pools opened before phase A (disjoint SBUF, allows
        # C x-prefetch to overlap phase A)
        xc = ctx.enter_context(tc.tile_pool(name="xc", bufs=3))
        oc = ctx.enter_context(tc.tile_pool(name="oc", bufs=3))

        # ---------------- Phase A: y = Wr @ x, stats ----------------
        with tc.tile_pool(name="xa", bufs=6) as xpool, \
             tc.tile_pool(name="cf", bufs=10) as cfpool, \
             tc.tile_pool(name="py", bufs=2, space="PSUM") as pypool, \
             tc.tile_pool(name="stg", bufs=4) as stgpool, \
             tc.tile_pool(name="sq", bufs=4) as sqpool:
            for j in range(NAIT):
                h0 = AH * j
                hl = AH * j + 1
                # ch-half 0: casting DMA (gpsimd); ch-half 1: f32 DMA + DVE cast
                c0 = cfpool.tile([128, 2, AH, W], F16, tag="cf")
                nc.gpsimd.dma_start(c0[:], xv[0:128, :, h0:h0 + AH, :])
                x1 = xpool.tile([128, 2, AH, W], F32, tag="xa")
                nc.sync.dma_start(x1[:], xv[128:256, :, h0:h0 + AH, :])
                c1f = cfpool.tile([128, 2, AH, W], F16, tag="cf")
                nc.vector.tensor_copy(c1f[:], x1[:])
                # psum tiles: [64, rp, 512-padded-bank], rp = row-pair
                pya = pypool.tile([64, 2, 512], F32, tag="pya")
                pyb = pypool.tile([64, 2, 512], F32, tag="pyb")
                for rp in range(2):
                    r0 = 2 * rp
                    nc.tensor.matmul(pya[:, rp, 0:320],
                                     wT0[:], c0[:, 0, r0:r0 + 2, :].rearrange("p r w -> p (r w)"),
                                     start=True, stop=False)
                    nc.tensor.matmul(pya[:, rp, 0:320],
                                     wT1[:], c1f[:, 0, r0:r0 + 2, :].rearrange("p r w -> p (r w)"),
                                     start=False, stop=True)
                    nc.tensor.matmul(pyb[:, rp, 0:320],
                                     wT0[:], c0[:, 1, r0:r0 + 2, :].rearrange("p r w -> p (r w)"),
                                     start=True, stop=False)
                    nc.tensor.matmul(pyb[:, rp, 0:320],
                                     wT1[:], c1f[:, 1, r0:r0 + 2, :].rearrange("p r w -> p (r w)"),
                                     start=False, stop=True)
                pav = pya[:, :, 0:320].rearrange("p a (r w) -> p a r w", r=2)
                pbv = pyb[:, :, 0:320].rearrange("p a (r w) -> p a r w", r=2)
                ydst = Y[0:64, hl:hl + AH, 1:161].rearrange("p (a r) w -> p a r w", a=2)
                # hb0: copy psum -> Y rows directly (+ sum accum) on DVE
                nc.vector.tensor_scalar(ydst, pav, 1.0, 0.0, ts.mult, ts.add,
                                        accum_out=sacc[:, j:j + 1])
                # hb1: copy psum -> staging (+ accum), then DMA into Y[64:128]
                stg = stgpool.tile([64, AH, W], F16, tag="stg")
                sv = stg[:].rearrange("p (a r) w -> p a r w", a=2)
                nc.vector.tensor_scalar(sv, pbv, 1.0, 0.0, ts.mult, ts.add,
                                        accum_out=sacc[:, NAIT + j:NAIT + j + 1])
                nc.sync.dma_start(Y[64:128, hl:hl + AH, 1:161], stg[:])
                # sum of squares (ACT from PSUM)
                sqa = sqpool.tile([64, 2, 320], F16, tag="sqa")
                nc.scalar.activation(sqa[:], pya[:, :, 0:320], AF.Square,
                                     accum_out=qacc[:, j:j + 1])
                sqb = sqpool.tile([64, 2, 320], F16, tag="sqb")
                nc.scalar.activation(sqb[:], pyb[:, :, 0:320], AF.Square,
                                     accum_out=qacc[:, NAIT + j:NAIT + j + 1])

        # halo rows (raw y): global row 80 -> hb0 hl81 ; global row 79 -> hb1 hl0
        nc.scalar.dma_start(Y[0:64, 81:82, :], Y[64:128, 1:2, :])
        nc.scalar.dma_start(Y[64:128, 0:1, :], Y[0:64, 80:81, :])

        # ---------------- stats + gate (tiny) ----------------
        with tc.tile_pool(name="stat", bufs=1) as stat, \
             tc.tile_pool(name="statp", bufs=1, space="PSUM") as statp:
            SQ = stat.tile([64, 2], F32, tag="SQ")
            nc.vector.tensor_reduce(SQ[:, 0:1], sacc[:], mybir.AxisListType.X, ts.add)
            nc.vector.tensor_reduce(SQ[:, 1:2], qacc[:], mybir.AxisListType.X, ts.add)
            ps8 = statp.tile([8, 2], F32, tag="ps8")
            nc.tensor.matmul(ps8[:], selg_sb[:], SQ[:], start=True, stop=True)

            mi = stat.tile([8, 2], F32, tag="mi")  # col0 mean, col1 invstd
            vtmp = stat.tile([8, 1], F32, tag="vtmp")
            npix_g = float(8 * NPIX)
            nc.vector.tensor_scalar(mi[:, 0:1], ps8[:, 0:1], 1.0 / npix_g, None,
                                    ts.mult)
            nc.vector.tensor_scalar(vtmp[:], ps8[:, 1:2], 1.0 / npix_g, None,
                                    ts.mult)
            msq = stat.tile([8, 1], F32, tag="msq")
            nc.vector.scalar_tensor_tensor(msq[:], mi[:, 0:1], 1.0, mi[:, 0:1],
                                           ts.bypass, ts.mult)
            nc.vector.tensor_sub(vtmp[:], vtmp[:], msq[:])
            nc.vector.tensor_scalar(vtmp[:], vtmp[:], GN_EPS, None, ts.add)
            nc.scalar.activation(vtmp[:], vtmp[:], AF.Sqrt)
            nc.vector.reciprocal(mi[:, 1:2], vtmp[:])
            mi128 = statp.tile([128, 2], F32, tag="mi128")
            nc.tensor.matmul(mi128[:], sel8_sb[:], mi[:], start=True, stop=True)

            # per-partition affine: A = invstd*scale ; Bc = bias - mean*A
            Acoef = persist.tile([128, 1], F32, tag="Acoef")
            Bcoef = persist.tile([128, 1], F32, tag="Bcoef")
            nc.vector.tensor_mul(Acoef[:], mi128[:, 1:2], gns_sb[:])
            tmpB = stat.tile([128, 1], F32, tag="tmpB")
            nc.vector.tensor_mul(tmpB[:], mi128[:, 0:1], Acoef[:])
            nc.vector.tensor_sub(Bcoef[:], gnb_sb[:], tmpB[:])

            # SE gate: p = mean(normalized y) per channel
            pgap = stat.tile([64, 1], F32, tag="pgap")
            nc.vector.tensor_scalar(pgap[:], SQ[:, 0:1], 1.0 / NPIX, None, ts.mult)
            nc.vector.scalar_tensor_tensor(pgap[:], pgap[:], Acoef[0:64, 0:1],
                                           Bcoef[0:64, 0:1], ts.mult, ts.add)
            hdn_ps = statp.tile([16, 1], F32, tag="hdn")
            nc.tensor.matmul(hdn_ps[:], w1T_sb[:], pgap[:], start=True, stop=True)
            hdn = stat.tile([16, 1], F32, tag="hdns")
            nc.scalar.activation(hdn[:], hdn_ps[:], AF.Relu, bias=b1_sb[:, 0:1])
            gam_ps = statp.tile([64, 1], F32, tag="gam")
            nc.tensor.matmul(gam_ps[:], w2T_sb[:], hdn[:], start=True, stop=True)
            gam = stat.tile([64, 1], F32, tag="gams")
            nc.scalar.activation(gam[:], gam_ps[:], AF.Sigmoid, bias=b2_sb[:, 0:1])
            # wc = fw1 + gamma*fw2  (fp16, both partition halves)
            wcf = stat.tile([64, 1], F32, tag="wcf")
            nc.vector.tensor_mul(wcf[:], gam[:], fw2_sb[:])
            nc.vector.tensor_add(wcf[:], wcf[:], fw1_sb[:])
            wcH = persist.tile([128, 1], F16, tag="wcH")
            nc.vector.tensor_copy(wcH[0:64, :], wcf[:])
            nc.scalar.dma_start(wcH[64:128, :], wcH[0:64, :])
            wsum_ps = statp.tile([1, 1], F32, tag="wsum_ps")
            nc.tensor.matmul(wsum_ps[:], wcH[0:64, :], ones64_sb[:],
                             start=True, stop=True)
            wsum = persist.tile([1, 1], F32, tag="wsum")
            nc.scalar.copy(wsum[:], wsum_ps[:])

            # B-chain runs on RAW y; normalization folds into ACT abs scales.
            # Pads must hold -B/A (the raw-space image of a zero-padded
            # normalized field).
            rA = stat.tile([128, 1], F32, tag="rA")
            nc.vector.reciprocal(rA[:], Acoef[:])
            pb = persist.tile([128, 1], F32, tag="pb")
            nc.vector.tensor_mul(pb[:], Bcoef[:], rA[:])
            nc.vector.tensor_scalar(pb[:], pb[:], -1.0, None, ts.mult)
            As05 = persist.tile([128, 1], F32, tag="As05")
            nc.vector.tensor_scalar(As05[:], Acoef[:], 0.5, None, ts.mult)
            As225 = persist.tile([128, 1], F32, tag="As225")
            nc.vector.tensor_scalar(As225[:], Acoef[:], 2.25, None, ts.mult)
            for pad, pbs in ((Y[:, :, 0:1], pb[:, 0:1]),
                             (Y[:, :, 161:162], pb[:, 0:1]),
                             (Y[0:64, 0:1, :], pb[0:64, 0:1]),
                             (Y[64:128, 81:82, :], pb[64:128, 0:1])):
                nc.gpsimd.memset(pad, 0.0)
                nc.vector.tensor_single_scalar(pad, pad, pbs, ts.add)
        if dbg and dbg.get("what") == "Y":
            nc.sync.dma_start(dbg_ext[:], Y[:])

        # ---------------- Phase B + C (pipelined over row chunks) ----------------
        with tc.tile_pool(name="bt", bufs=1) as bt, \
             tc.tile_pool(name="tchunk", bufs=4) as tpool, \
             tc.tile_pool(name="gsg", bufs=8) as gsg, \
             tc.tile_pool(name="sps", bufs=4, space="PSUM") as sps, \
             tc.tile_pool(name="gps", bufs=4, space="PSUM") as gps:
            for k in range(NCHUNK):
                a = 1 + CH * k
                c1a = bt.tile([128, CH, WP], F16, tag="c1a")
                nc.vector.tensor_add(c1a[:], Y[:, a - 1:a + CH - 1, :], Y[:, a:a + CH, :])
                c1 = bt.tile([128, CH, WP], F16, tag="c1")
                nc.vector.tensor_add(c1[:], c1a[:], Y[:, a + 1:a + CH + 1, :])
                dv = bt.tile([128, CH, WP], F16, tag="dv")
                nc.gpsimd.tensor_sub(dv[:], Y[:, a - 1:a + CH - 1, :], Y[:, a + 1:a + CH + 1, :])
                av = bt.tile([128, CH, WP], F16, tag="av")
                nc.vector.tensor_add(av[:], c1[:], Y[:, a:a + CH, :])
                ma = bt.tile([128, CH, WP], F16, tag="ma")
                nc.vector.tensor_add(ma[:, :, 0:161], c1[:, :, 0:161], c1[:, :, 1:162])
                m9 = bt.tile([128, CH, W], F16, tag="m9")
                nc.vector.tensor_add(m9[:], ma[:, :, 0:160], c1[:, :, 2:162])
                gx4 = bt.tile([128, CH, W], F16, tag="gx4")
                nc.vector.tensor_sub(gx4[:], av[:, :, 0:160], av[:, :, 2:162])
                e1 = bt.tile([128, CH, WP], F16, tag="e1")
                nc.gpsimd.tensor_add(e1[:, :, 0:161], dv[:, :, 0:161], dv[:, :, 1:162])
                gy4 = bt.tile([128, CH, W], F16, tag="gy4")
                nc.vector.tensor_add(gy4[:], e1[:, :, 0:160], e1[:, :, 1:161])
                y9 = bt.tile([128, CH, W], F16, tag="y9")
                nc.vector.tensor_scalar(y9[:], Y[:, a:a + CH, 1:161], 9.0, None,
                                        ts.mult)
                d9 = bt.tile([128, CH, W], F16, tag="d9")
                nc.vector.tensor_sub(d9[:], y9[:], m9[:])
                # n9h = 0.5*|9y - m|
                n9h = bt.tile([128, CH, W], F16, tag="n9h")
                nc.scalar.activation(n9h[:], d9[:], AF.Abs, scale=As05[:, 0:1])
                ax2 = bt.tile([128, CH, W], F16, tag="ax2")
                nc.scalar.activation(ax2[:], gx4[:], AF.Abs, scale=As225[:, 0:1])
                ay2 = bt.tile([128, CH, W], F16, tag="ay2")
                nc.scalar.activation(ay2[:], gy4[:], AF.Abs, scale=As225[:, 0:1])
                q9 = bt.tile([128, CH, W], F16, tag="q9")
                nc.vector.tensor_add(q9[:], ax2[:], ay2[:])
                Ef = bt.tile([128, CH, W], F32, tag="Ef")
                nc.vector.scalar_tensor_tensor(Ef[:], q9[:], 9.0 * EPS, n9h[:],
                                               ts.add, ts.max)
                rE = bt.tile([128, CH, W], F32, tag="rE")
                nc.vector.reciprocal_approx_fast(rE[:], Ef[:])
                # t/2 = 0.5 * min(ratio, 2); the missing 2x is folded into the
                # sigmoid scale below.
                tch = tpool.tile([128, CH, W], F16, tag="tch")
                nc.vector.tensor_mul(tch[:], n9h[:], rE[:])
                if dbg and dbg.get("what") == "t":
                    nc.sync.dma_start(dbg_ext[:, CH * k:CH * (k + 1), :], tch[:])

                # ---- phase C for this chunk ----
                for hh, (hoff, HC) in enumerate(((0, CH),)):
                    hc = CH * k + hoff
                    xt0 = xc.tile([128, 2, HC, W], F32, tag="xc")
                    nc.sync.dma_start(xt0[:], xv[0:128, :, hc:hc + HC, :])
                    xt1 = xc.tile([128, 2, HC, W], F32, tag="xc")
                    nc.sync.dma_start(xt1[:], xv[128:256, :, hc:hc + HC, :])
                    ot0 = oc.tile([128, 2, HC, W], F32, tag="oc")
                    ot1 = oc.tile([128, 2, HC, W], F32, tag="oc")
                    sbase = hoff
                    for s2 in range(HC // 2):
                        r2loc = 2 * s2
                        r2 = sbase + r2loc
                        s0 = sps.tile([1, 320], F32, tag="sfuse")
                        s1 = sps.tile([1, 320], F32, tag="sfuse")
                        r0 = tch[0:64, r2:r2 + 2, :].rearrange("p r w -> p (r w)")
                        r1 = tch[64:128, r2:r2 + 2, :].rearrange("p r w -> p (r w)")
                        nc.tensor.matmul(s0[:], wcH[0:64, :], r0, start=True, stop=True)
                        nc.tensor.matmul(s1[:], wcH[64:128, :], r1, start=True, stop=True)
                        g0 = gsg.tile([1, 320], F16, tag="gsig")
                        g1 = gsg.tile([1, 320], F16, tag="gsig")
                        # a = sigmoid(sum(wc) - 2*sum(wc * t/2))
                        nc.scalar.activation(g0[:], s0[:], AF.Sigmoid,
                                             bias=wsum[0:1, 0:1], scale=-2.0)
                        nc.scalar.activation(g1[:], s1[:], AF.Sigmoid,
                                             bias=wsum[0:1, 0:1], scale=-2.0)
                        G0 = gps.tile([128, 320], F32, tag="G")
                        G1 = gps.tile([128, 320], F32, tag="G")
                        nc.tensor.matmul(G0[:], tenth_sb[:], g0[:], start=True, stop=False)
                        nc.tensor.matmul(G0[:], ones_sb[:], onesrow_sb[:], start=False, stop=True)
                        nc.tensor.matmul(G1[:], tenth_sb[:], g1[:], start=True, stop=False)
                        nc.tensor.matmul(G1[:], ones_sb[:], onesrow_sb[:], start=False, stop=True)
                        G1s = gsg.tile([128, 320], F32, tag="G1s")
                        nc.scalar.copy(G1s[:], G1[:])
                        for (xt, ot) in ((xt0, ot0), (xt1, ot1)):
                            nc.vector.tensor_mul(
                                ot[:, 0, r2loc:r2loc + 2, :].rearrange("p r w -> p (r w)"),
                                xt[:, 0, r2loc:r2loc + 2, :].rearrange("p r w -> p (r w)"), G0[:])
                            nc.gpsimd.tensor_mul(
                                ot[:, 1, r2loc:r2loc + 2, :].rearrange("p r w -> p (r w)"),
                                xt[:, 1, r2loc:r2loc + 2, :].rearrange("p r w -> p (r w)"), G1s[:])
                    nc.scalar.dma_start(ov[0:128, :, hc:hc + HC, :], ot0[:])
                    nc.scalar.dma_start(ov[128:256, :, hc:hc + HC, :], ot1[:])


_NC_CACHE = {}


def _get_nc():
    if "nc" not in _NC_CACHE:
        _NC_CACHE["nc"] = build_kernel()
    return _NC_CACHE["nc"]


def kernel(x, reduce_w, gn_scale, gn_bias, gate_w1, gate_b1, gate_w2, gate_b2,
           fuse_w):
    x = np.ascontiguousarray(np.asarray(x, np.float32))
    rwT = np.ascontiguousarray(
        np.asarray(reduce_w, np.float32)[:, :, 0, 0].T.astype(np.float16))
    w1T = np.ascontiguousarray(np.asarray(gate_w1, np.float32)[:, :, 0, 0].T)
    w2T = np.ascontiguousarray(np.asarray(gate_w2, np.float32)[:, :, 0, 0].T)
    b1 = np.asarray(gate_b1, np.float32).reshape(16, 1)
    b2 = np.asarray(gate_b2, np.float32).reshape(64, 1)
    gns = np.ascontiguousarray(np.tile(np.asarray(gn_scale, np.float32), 2).reshape(128, 1))
    gnb = np.ascontiguousarray(np.tile(np.asarray(gn_bias, np.float32), 2).reshape(128, 1))
    fw = np.asarray(fuse_w, np.float32)[0, :, 0, 0]
    fw1 = np.ascontiguousarray(fw[:CRED].reshape(64, 1))
    fw2 = np.ascontiguousarray(fw[CRED:].reshape(64, 1))

    nc = _get_nc()
    shared = dict(rwT=rwT, w1T=w1T, b1=b1, w2T=w2T, b2=b2, gns=gns, gnb=gnb,
                  fw1=fw1, fw2=fw2)
    in_maps = [dict(x=np.ascontiguousarray(x[i]), **shared) for i in range(B)]
    res = run_bass_kernel_spmd(nc, in_maps, core_ids=list(range(8)))
    return np.stack([res.results[i]["out"] for i in range(B)], axis=0)
